# revision 14
# baseline (speedup 1.0000x reference)
"""Self-contained Trainium2 Bass kernel for nn_MultiHeadAttn_49357764166084.

Sharding: 8 cores = 2 batches x 4 head-groups (4 heads each).
Per core: LN-folded QKV projections (fp32r matmuls), S^T = K Q^T scores,
AV = V^T S^T, partial out = O^T.T @ Wo_rows. Host sums Wo partials +
residual and interleaves per-core [j, i, 4] score blocks into
attn_prob [i, j, b, n].
"""
import os
import sys
import types

import numpy as np

import concourse.bass as bass
import concourse.mybir as mybir
import concourse.tile as tile
from contextlib import ExitStack
from concourse.bass_utils import run_bass_kernel_spmd
from concourse.masks import make_identity

T = 2048
B = 2
D = 1024
NH = 16
DH = 64
E = 256            # 4 heads * 64 per core
SCALE = 1.0 / (DH ** 0.5)
LN_EPS = 1e-5
MASK_VAL = float(np.finfo(np.float32).min)

F32 = mybir.dt.float32
F32R = mybir.dt.float32r
MULT = None  # set after import
ADD = None

LAST_EXEC_TIME_NS = None


def _install_ntff_shim():
    """Register the axon NTFF profile hook if the image's antenv lacks it."""
    if "antenv.axon_hooks" in sys.modules:
        return
    try:
        import antenv
        mod = types.ModuleType("antenv.axon_hooks")
        _hook = [None]
        mod.set_axon_ntff_profile_hook = lambda h: _hook.__setitem__(0, h)
        mod.get_axon_ntff_profile_hook = lambda: _hook[0]
        sys.modules["antenv.axon_hooks"] = mod
        antenv.axon_hooks = mod
        from trn_agent_boot.trn_boot import _ntff_profile_via_ctypes
        mod.set_axon_ntff_profile_hook(
            _ntff_profile_via_ctypes("/opt/axon/libaxon_pjrt.so"))
    except Exception:
        pass


def _split_excess_waits(nc, max_waits=1):
    """Walrus rejects instructions with >~1 sem wait; move overflow to NOPs."""
    for bb in nc.m.functions[0].blocks:
        newlist = []
        for inst in bb.instructions:
            si = inst.sync_info
            if si is not None and si.on_wait and len(si.on_wait) > max_waits:
                waits = list(si.on_wait)
                overflow, keep = waits[:-max_waits], waits[-max_waits:]
                for i, w in enumerate(overflow):
                    nop = mybir.InstNoOp(name=f"{inst.name}-ws{i}", ins=[], outs=[])
                    nop.engine = inst.engine
                    nop.sync_info = mybir.SyncInfo(on_wait=[w], on_update=[])
                    newlist.append(nop)
                inst.sync_info = mybir.SyncInfo(
                    on_wait=keep, on_update=list(si.on_update or []))
            newlist.append(inst)
        bb.instructions[:] = newlist


def build_nc():
    mult = mybir.AluOpType.mult
    add = mybir.AluOpType.add
    sub = mybir.AluOpType.subtract

    nc = bass.Bass("TRN2", target_bir_lowering=False, debug=False)

    # ---- DRAM I/O ----
    hb = nc.dram_tensor("hb", [T, D], F32, kind="ExternalInput").ap()
    wq = nc.dram_tensor("wq", [D, E], F32R, kind="ExternalInput").ap()
    wk = nc.dram_tensor("wk", [D, E], F32R, kind="ExternalInput").ap()
    wv = nc.dram_tensor("wv", [D, E], F32R, kind="ExternalInput").ap()
    wo = nc.dram_tensor("wo", [64, 4 * D], F32R, kind="ExternalInput").ap()
    # per-partition-layout aux vectors (host pre-tiled)
    mscd = nc.dram_tensor("msc", [128, 16], F32, kind="ExternalInput").ap()
    mbsd = nc.dram_tensor("mbs", [128, 16], F32, kind="ExternalInput").ap()
    wk2d = nc.dram_tensor("wk2", [128, 2], F32, kind="ExternalInput").ap()
    bk2d = nc.dram_tensor("bk2", [128, 2], F32, kind="ExternalInput").ap()
    wvsd = nc.dram_tensor("wvs", [E], F32, kind="ExternalInput").ap()
    bvsd = nc.dram_tensor("bvs", [E], F32, kind="ExternalInput").ap()

    scores = nc.dram_tensor("scores", [T, T, 4], F32R, kind="ExternalOutput").ap()
    outp = nc.dram_tensor("outp", [T, D], F32, kind="ExternalOutput").ap()

    with tile.TileContext(nc) as tc:
        with ExitStack() as ctx:
            # ---------- persistent pool ----------
            per = ctx.enter_context(tc.tile_pool(name="per", bufs=1))
            dram = ctx.enter_context(tc.tile_pool(name="dram", bufs=1, space="DRAM"))

            ident = per.tile([128, 128], F32)
            make_identity(nc, ident[:])
            eps_t = per.tile([128, 1], F32)
            nc.gpsimd.memset(eps_t[:], LN_EPS)

            wq_sb = per.tile([128, 8 * E], F32R)
            wk_sb = per.tile([128, 8 * E], F32R)
            wv_sb = per.tile([128, 8 * E], F32R)
            wo_sb = per.tile([64, 4 * D], F32R)
            for k in range(8):
                nc.sync.dma_start(wq_sb[:, k * E:(k + 1) * E], wq[k * 128:(k + 1) * 128, :])
                nc.sync.dma_start(wk_sb[:, k * E:(k + 1) * E], wk[k * 128:(k + 1) * 128, :])
                nc.sync.dma_start(wv_sb[:, k * E:(k + 1) * E], wv[k * 128:(k + 1) * 128, :])
            nc.sync.dma_start(wo_sb[:], wo[:])

            msc = per.tile([128, 16], F32)
            mbs = per.tile([128, 16], F32)
            wk2 = per.tile([128, 2], F32)
            bk2 = per.tile([128, 2], F32)
            wv_b = per.tile([128, E], F32)
            bv_b = per.tile([128, E], F32)
            nc.sync.dma_start(msc[:], mscd[:])
            nc.sync.dma_start(mbs[:], mbsd[:])
            nc.sync.dma_start(wk2[:], wk2d[:])
            nc.sync.dma_start(bk2[:], bk2d[:])
            nc.sync.dma_start(wv_b[:], wvsd[None, :].broadcast_to((128, E)))
            nc.sync.dma_start(bv_b[:], bvsd[None, :].broadcast_to((128, E)))

            qa = per.tile([128, T], F32R)   # Q^T heads 0-1
            qb = per.tile([128, T], F32R)   # Q^T heads 2-3
            ka = per.tile([128, T], F32R)
            kb = per.tile([128, T], F32R)
            v_sb = per.tile([128, 16 * E], F32R)   # V[t, e] as 16 j-tiles
            rstd_all = per.tile([128, 16], F32)
            m2_all = per.tile([128, 16], F32)
            agg = per.tile([128, 2], F32)
            stdv = per.tile([128, 1], F32)
            st6 = per.tile([128, 12], F32)

            rstd_dram = dram.tile([T], F32)
            m2_dram = dram.tile([T], F32)

            # ---------- phase 1: load h, stats, transpose, projections ----------
            with ExitStack() as p1:
                ht_pool = p1.enter_context(tc.tile_pool(name="ht", bufs=1))
                p1sb = p1.enter_context(tc.tile_pool(name="p1sb", bufs=1))
                rstd_b = p1sb.tile([128, T], F32)
                m2_b = p1sb.tile([128, T], F32)
                hn_pool = p1.enter_context(tc.tile_pool(name="hn", bufs=3))
                pst = p1.enter_context(tc.tile_pool(name="pst", bufs=3, space="PSUM"))
                psp = p1.enter_context(tc.tile_pool(name="psp", bufs=2, space="PSUM"))
                psv = p1.enter_context(tc.tile_pool(name="psv", bufs=2, space="PSUM"))

                ht = ht_pool.tile([128, 8 * T], F32R)   # h^T: 8 d-tiles x [128, 2048]
                ht_v = ht[:].rearrange("p (k t) -> p k t", t=T)

                for tt in range(16):
                    hn = hn_pool.tile([128, D], F32, tag="hn")
                    nc.sync.dma_start(hn[:], hb[tt * 128:(tt + 1) * 128, :])
                    # LN stats
                    nc.vector.bn_stats(st6[:, 0:6], hn[:, 0:512])
                    nc.vector.bn_stats(st6[:, 6:12], hn[:, 512:1024])
                    nc.vector.bn_aggr(agg[:], st6[:])
                    nc.scalar.activation(stdv[:], agg[:, 1:2],
                                         mybir.ActivationFunctionType.Sqrt,
                                         bias=eps_t[:])
                    nc.vector.reciprocal(rstd_all[:, tt:tt + 1], stdv[:])
                    nc.vector.tensor_mul(m2_all[:, tt:tt + 1], agg[:, 0:1],
                                         rstd_all[:, tt:tt + 1])
                    # transpose 8 d-blocks -> psum, evac to ht (ACT, casts f32r)
                    for half in range(2):
                        pt = pst.tile([128, 512], F32, tag="pt")
                        for k in range(4):
                            nc.tensor.transpose(pt[:, k * 128:(k + 1) * 128],
                                                hn[:, (half * 4 + k) * 128:(half * 4 + k + 1) * 128],
                                                ident[:])
                        dst = ht_v[:, half * 4:half * 4 + 4, tt * 128:(tt + 1) * 128]
                        src = pt[:].rearrange("p (k t) -> p k t", t=128)
                        nc.scalar.copy(dst, src)

                # mu/rstd rows -> HBM -> broadcast tiles
                for tt in range(16):
                    nc.sync.dma_start(rstd_dram[tt * 128:(tt + 1) * 128],
                                      rstd_all[:, tt:tt + 1])
                    nc.sync.dma_start(m2_dram[tt * 128:(tt + 1) * 128],
                                      m2_all[:, tt:tt + 1])
                nc.sync.dma_start(rstd_b[:], rstd_dram[None, :].broadcast_to((128, T)))
                nc.sync.dma_start(m2_b[:], m2_dram[None, :].broadcast_to((128, T)))

                # Q^T / K^T projections: out[e_half, t-chunk]
                for half, (qdst, kdst) in enumerate(((qa, ka), (qb, kb))):
                    for tc4 in range(4):
                        sl = slice(tc4 * 512, (tc4 + 1) * 512)
                        pq = psp.tile([128, 512], F32, tag="pq")
                        for k in range(8):
                            nc.tensor.matmul(
                                pq[:],
                                wq_sb[:, k * E + half * 128: k * E + half * 128 + 128],
                                ht_v[:, k, sl],
                                start=(k == 0), stop=(k == 7))
                        nc.scalar.copy(qdst[:, sl], pq[:])
                        pk = psp.tile([128, 512], F32, tag="pq")
                        for k in range(8):
                            nc.tensor.matmul(
                                pk[:],
                                wk_sb[:, k * E + half * 128: k * E + half * 128 + 128],
                                ht_v[:, k, sl],
                                start=(k == 0), stop=(k == 7))
                        # K^T = P*rstd_b - ((m2_b*wksum) - bk)
                        u2k = p1sb.tile([128, 512], F32, tag="u2k", name="u2k")
                        k1 = p1sb.tile([128, 512], F32, tag="k1tmp", name="k1")
                        nc.vector.scalar_tensor_tensor(
                            u2k[:], m2_b[:, sl], wk2[:, half:half + 1],
                            bk2[:, half:half + 1].broadcast_to((128, 512)),
                            op0=mult, op1=sub)
                        nc.vector.tensor_mul(k1[:], pk[:], rstd_b[:, sl])
                        nc.vector.tensor_sub(kdst[:, sl], k1[:], u2k[:])

                # V projection: out[j-tile, e]
                for jt in range(16):
                    pv = psv.tile([128, E], F32, tag="pv")
                    for k in range(8):
                        nc.tensor.matmul(
                            pv[:],
                            ht_v[:, k, jt * 128:(jt + 1) * 128],
                            wv_sb[:, k * E:(k + 1) * E],
                            start=(k == 0), stop=(k == 7))
                    u2v = p1sb.tile([128, E], F32, tag="u2v", name="u2v")
                    nc.vector.scalar_tensor_tensor(
                        u2v[:], wv_b[:], m2_all[:, jt:jt + 1], bv_b[:],
                        op0=mult, op1=sub)
                    nc.vector.scalar_tensor_tensor(
                        v_sb[:, jt * E:(jt + 1) * E], pv[:],
                        rstd_all[:, jt:jt + 1], u2v[:],
                        op0=mult, op1=sub)

            # O^T tensors live through phases 2+3; allocate after phase-1 frees
            late = ctx.enter_context(tc.tile_pool(name="late", bufs=1))
            o_h = [late.tile([64, T], F32R, tag=f"o{hh}", name=f"o{hh}") for hh in range(4)]

            # ---------- phase 2: scores + AV ----------
            with ExitStack() as p2:
                stc_pool = p2.enter_context(tc.tile_pool(name="stc", bufs=3))
                sti_pool = p2.enter_context(tc.tile_pool(name="sti", bufs=3))
                ps_s = p2.enter_context(tc.tile_pool(name="ps_s", bufs=2, space="PSUM"))
                ps_av = p2.enter_context(tc.tile_pool(name="ps_av", bufs=1, space="PSUM"))

                for iq in range(4):
                    isl = slice(iq * 512, (iq + 1) * 512)
                    av = [ps_av.tile([64, 512], F32, tag=f"av{hh}", name=f"av{hh}") for hh in range(4)]
                    for jt in range(16):
                        jsl = slice(jt * 128, (jt + 1) * 128)
                        sp01 = ps_s.tile([128, 1024], F32, tag="sp")
                        sp23 = ps_s.tile([128, 1024], F32, tag="sp")
                        for hh in range(4):
                            sp = sp01 if hh < 2 else sp23
                            col = (hh % 2) * 512
                            kt = ka if hh < 2 else kb
                            qt = qa if hh < 2 else qb
                            bp = 64 * (hh % 2)
                            nc.tensor.matmul(
                                sp[:, col:col + 512],
                                kt[bp:bp + 64, jsl],
                                qt[bp:bp + 64, isl],
                                start=True, stop=True)
                        # pass A (DVE): mask affine, to f32r staging
                        stc = stc_pool.tile([128, 2048], F32R, tag="stc")
                        nc.vector.tensor_scalar(
                            stc[:, 0:1024], sp01[:],
                            msc[:, jt:jt + 1], mbs[:, jt:jt + 1],
                            op0=mult, op1=add)
                        nc.vector.tensor_scalar(
                            stc[:, 1024:2048], sp23[:],
                            msc[:, jt:jt + 1], mbs[:, jt:jt + 1],
                            op0=mult, op1=add)
                        # pass B (ACT): interleave (i, hh)
                        sti = sti_pool.tile([128, 2048], F32R, tag="sti")
                        nc.scalar.copy(
                            sti[:].rearrange("p (i hh) -> p i hh", hh=4),
                            stc[:].rearrange("p (hh i) -> p i hh", hh=4))
                        # AV accumulate
                        for hh in range(4):
                            nc.tensor.matmul(
                                av[hh][:],
                                v_sb[:, jt * E + hh * 64: jt * E + hh * 64 + 64],
                                stc[:, hh * 512:(hh + 1) * 512],
                                start=(jt == 0), stop=(jt == 15))
                        # scores out
                        nc.sync.dma_start(
                            scores[jsl, isl, :].rearrange("p i hh -> p (i hh)"),
                            sti[:])
                    for hh in range(4):
                        nc.scalar.copy(o_h[hh][:, isl], av[hh][:])

            # ---------- phase 3: Wo ----------
            with ExitStack() as p3:
                out_pool = p3.enter_context(tc.tile_pool(name="outp", bufs=2))
                ps_o = p3.enter_context(tc.tile_pool(name="ps_o", bufs=2, space="PSUM"))
                for tt in range(16):
                    tsl = slice(tt * 128, (tt + 1) * 128)
                    osb = out_pool.tile([128, D], F32, tag="osb")
                    for nch in range(2):
                        po = ps_o.tile([128, 512], F32, tag="po")
                        for hh in range(4):
                            nc.tensor.matmul(
                                po[:],
                                o_h[hh][:, tsl],
                                wo_sb[:, hh * D + nch * 512: hh * D + nch * 512 + 512],
                                start=(hh == 0), stop=(hh == 3))
                        if nch == 0:
                            nc.vector.tensor_copy(osb[:, 0:512], po[:])
                        else:
                            nc.scalar.copy(osb[:, 512:1024], po[:])
                    nc.sync.dma_start(outp[tsl, :], osb[:])

    _split_excess_waits(nc)
    return nc


_NC_CACHE = None


def kernel(h, Wq, Wkv, Wo, gamma, beta, attn_mask):
    global _NC_CACHE, LAST_EXEC_TIME_NS
    _install_ntff_shim()

    h = np.asarray(h, dtype=np.float32)
    Wq = np.asarray(Wq, dtype=np.float32)
    Wkv = np.asarray(Wkv, dtype=np.float32)
    Wo = np.asarray(Wo, dtype=np.float32)
    gamma = np.asarray(gamma, dtype=np.float32)
    beta = np.asarray(beta, dtype=np.float32)
    attn_mask = np.asarray(attn_mask)

    if _NC_CACHE is None:
        _NC_CACHE = build_nc()
    nc = _NC_CACHE

    Wk_full = Wkv[:, :NH * DH]
    Wv_full = Wkv[:, NH * DH:]

    in_maps = []
    for c in range(8):
        b, g = c // 4, c % 4
        cols = slice(g * E, (g + 1) * E)
        wq_s = np.ascontiguousarray(Wq[:, cols] * SCALE, dtype=np.float32)
        wk_s = np.ascontiguousarray(gamma[:, None] * Wk_full[:, cols], dtype=np.float32)
        wv_s = np.ascontiguousarray(gamma[:, None] * Wv_full[:, cols], dtype=np.float32)
        wo_rows = Wo[g * E:(g + 1) * E, :]
        wo_s = np.ascontiguousarray(
            wo_rows.reshape(4, 64, D).transpose(1, 0, 2).reshape(64, 4 * D),
            dtype=np.float32)
        bk = (beta @ Wk_full[:, cols]).astype(np.float32)
        bv = (beta @ Wv_full[:, cols]).astype(np.float32)
        wks = wk_s.sum(axis=0).astype(np.float32)
        wvs = wv_s.sum(axis=0).astype(np.float32)
        m = attn_mask[:, b].astype(np.float32)
        mscale = (1.0 - m).astype(np.float32)
        mbias = (m * MASK_VAL).astype(np.float32)
        in_maps.append({
            "hb": np.ascontiguousarray(h[:, b, :]),
            "wq": wq_s, "wk": wk_s, "wv": wv_s, "wo": wo_s,
            "msc": np.ascontiguousarray(mscale.reshape(16, 128).T),
            "mbs": np.ascontiguousarray(mbias.reshape(16, 128).T),
            "wk2": np.ascontiguousarray(wks.reshape(2, 128).T),
            "bk2": np.ascontiguousarray(bk.reshape(2, 128).T),
            "wvs": wvs, "bvs": bv,
        })

    trace = bool(os.environ.get("KERNEL_TRACE"))
    res = run_bass_kernel_spmd(nc, in_maps, core_ids=list(range(8)), trace=trace)
    LAST_EXEC_TIME_NS = res.exec_time_ns

    attn_prob = np.empty((T, T, B, NH), dtype=np.float32)
    out = np.empty((T, B, D), dtype=np.float32)
    acc = [np.zeros((T, D), dtype=np.float64) for _ in range(B)]
    for c in range(8):
        b, g = c // 4, c % 4
        r = res.results[c]
        attn_prob[:, :, b, g * 4:(g + 1) * 4] = r["scores"].transpose(1, 0, 2)
        acc[b] += r["outp"].astype(np.float64)
    for b in range(B):
        out[:, b, :] = h[:, b, :] + acc[b].astype(np.float32)
    return out, attn_prob
